# revision 1
# baseline (speedup 1.0000x reference)
"""Trainium2 Bass kernel for MeshGNN message passing (8 NeuronCores, SPMD).

Math reformulation (exact): since softmax weights sum to 1 and the output MLP is
linear, fold W_concat/W_out into per-node quantities:
    M1 = W_out @ W_concat[:, :128]   [3,128]
    M2 = W_out @ W_concat[:, 128:]   [3,3]
    c0 = b_concat @ W_out.T + b_out  [3]
    kx[j] = x[j] @ W_k.T + b_k                  (64,)   -> table
    w[j]  = x[j] @ M1.T + p[j] @ M2.T           (3,)    -> table
    q[n]  = (x[n] @ W_q.T + b_q) / scale        (64,)
    scores[n,k] = q[n] . kx[nbr]
    e = exp(scores * (nbr != 0))                         (scores bounded ~±3)
    out[n] = p[n] + (-v[n] + c0) + sum_k e_k * w[nbr] / sum_k e_k,  v = p @ M2.T

Implementation: per-node table rows of 128 fp16 (=256B): [kx(64)|w(3)|pad61].
Rows are fetched with dma_gather in PAIRS (512B, idx = nbr//2 fits int16),
and the correct half is selected arithmetically via host-prepared parity
masks folded into the score/softmax math (multi-row indirect_dma_start is
broken on HW; dma_gather is the production batched-gather path).
Phase 1 computes the fp16 table + q + base per 128-node tile with one matmul;
an AllGather shares the table; phase 2 gathers pairs chunk-wise and runs the
attention on DVE/ACT.
"""

import sys

import numpy as np

sys.path.insert(0, "/opt/trn_rl_repo")

import concourse.bass as bass
import concourse.mybir as mybir
import concourse.tile as tile
from concourse import bacc
from concourse.bass import ds, ts
from concourse.bass_utils import run_bass_kernel_spmd

N_CORES = 8
H = 128
K = 15
DT = mybir.dt
F16 = DT.float16
F32 = DT.float32
I16 = DT.int16

ROW = 128                 # fp16 elems per table row (256B)
PAIR = 2 * ROW            # gather element: two rows (512B)
QC = 64                   # q/k dim
W1C = 131                 # matmul cols: kx(64)|w(3)|q(64)


def build_program(n_total, shard, n_tiles, chunk_tiles=3):
    valid = n_total // N_CORES
    P = 128
    nc = bacc.Bacc(None, debug=False, num_swdge_queues=4)

    ax = nc.declare_dram_parameter("ax", [P, shard], F16, isOutput=False)    # x.T
    pts = nc.declare_dram_parameter("pts", [P, n_tiles * 3], F32, isOutput=False)
    idx16 = nc.declare_dram_parameter("idx16", [P, n_tiles * K * 8], I16,
                                      isOutput=False)
    cmask = nc.declare_dram_parameter("cmask", [P, n_tiles * K * 2], F32,
                                      isOutput=False)
    pmask = nc.declare_dram_parameter("pmask", [P, n_tiles * K * 2], F16,
                                      isOutput=False)
    w1 = nc.declare_dram_parameter("w1", [P, W1C], F16, isOutput=False)
    brow = nc.declare_dram_parameter("brow", [P, W1C], F16, isOutput=False)
    c0r = nc.declare_dram_parameter("c0r", [P, 3], F32, isOutput=False)
    m2r = nc.declare_dram_parameter("m2r", [P, 9], F32, isOutput=False)
    out = nc.declare_dram_parameter("out", [P, n_tiles * 3], F32, isOutput=True)

    with tile.TileContext(nc) as tc:
        with (
            tc.tile_pool(name="persist", bufs=1) as pp,
            tc.tile_pool(name="dram", bufs=1, space="DRAM") as dp,
            tc.tile_pool(name="psum", bufs=4, space="PSUM") as psp,
            tc.tile_pool(name="kxgp", bufs=3) as kxgp,
            tc.tile_pool(name="work", bufs=2) as wp,
        ):
            # ---- persistent SBUF ----
            xT = pp.tile([P, shard], F16)
            pts_sb = pp.tile([P, n_tiles * 3], F32)
            idx_sb = pp.tile([P, n_tiles * K * 8], I16)
            cm_sb = pp.tile([P, n_tiles * K * 2], F32)
            pm_sb = pp.tile([P, n_tiles * K * 2], F16)
            w1_sb = pp.tile([P, W1C], F16)
            br_sb = pp.tile([P, W1C], F16)
            c0_sb = pp.tile([P, 3], F32)
            m2_sb = pp.tile([P, 9], F32)
            q_sb = pp.tile([P, n_tiles * QC], F16)
            base_sb = pp.tile([P, n_tiles * 3], F32)
            out_sb = pp.tile([P, n_tiles * 3], F32)
            tblall_sb = pp.tile([P, n_tiles * ROW], F16)
            stage_sb = pp.tile([P, n_tiles * W1C], F16)

            table_pad = dp.tile([shard, ROW], F16, space="DRAM")
            table_full = dp.tile([n_total, ROW], F16, space="DRAM",
                                 addr_space="Shared")

            nc.sync.dma_start(out=xT[:], in_=ax[:, :])
            nc.sync.dma_start(out=pts_sb[:], in_=pts[:, :])
            nc.sync.dma_start(out=idx_sb[:], in_=idx16[:, :])
            nc.sync.dma_start(out=cm_sb[:], in_=cmask[:, :])
            nc.sync.dma_start(out=pm_sb[:], in_=pmask[:, :])
            nc.sync.dma_start(out=w1_sb[:], in_=w1[:, :])
            nc.sync.dma_start(out=br_sb[:], in_=brow[:, :])
            nc.sync.dma_start(out=c0_sb[:], in_=c0r[:, :])
            nc.sync.dma_start(out=m2_sb[:], in_=m2r[:, :])

            nc.vector.memset(tblall_sb[:], 0)

            # ---- phase 1: matmuls -> ACT-staged copies -> batched DVE ----
            NT = n_tiles
            for t in range(n_tiles):
                ps = psp.tile([P, W1C], F32, space="PSUM", tag="ps")
                nc.tensor.matmul(out=ps[:], lhsT=xT[:, ts(t, P)], rhs=w1_sb[:],
                                 start=True, stop=True)
                nc.scalar.copy(out=stage_sb[:, ts(t, W1C)], in_=ps[:])

            st3 = stage_sb[:].rearrange("p (t c) -> p t c", c=W1C)
            # v = p @ M2.T for all tiles: [P, NT, 3]
            vp_all = pp.tile([P, NT * 9], F32)
            for j in range(3):
                nc.vector.tensor_tensor(
                    out=vp_all[:].rearrange("p (t j i) -> p t j i", j=3, i=3)
                        [:, :, j, :],
                    in0=pts_sb[:].rearrange("p (t i) -> p t i", i=3),
                    in1=m2_sb[:, ds(3 * j, 3)].unsqueeze(1)
                        .broadcast_to([P, NT, 3]),
                    op=mybir.AluOpType.mult)
            v_all = pp.tile([P, NT * 3], F32)
            nc.vector.tensor_reduce(
                out=v_all[:],
                in_=vp_all[:].rearrange("p (t j i) -> p (t j) i", j=3, i=3),
                axis=mybir.AxisListType.X, op=mybir.AluOpType.add)
            v16 = pp.tile([P, NT * 3], F16)
            nc.vector.tensor_copy(out=v16[:], in_=v_all[:])
            tbl3 = tblall_sb[:].rearrange("p (t e) -> p t e", e=ROW)
            nc.vector.tensor_tensor(
                out=tbl3[:, :, 0:67], in0=st3[:, :, 0:67],
                in1=br_sb[:, 0:67].unsqueeze(1).broadcast_to([P, NT, 67]),
                op=mybir.AluOpType.add)
            nc.vector.tensor_tensor(
                out=tbl3[:, :, 64:67], in0=tbl3[:, :, 64:67],
                in1=v16[:].rearrange("p (t i) -> p t i", i=3),
                op=mybir.AluOpType.add)
            nc.vector.tensor_tensor(
                out=q_sb[:].rearrange("p (t e) -> p t e", e=QC),
                in0=st3[:, :, 67:W1C],
                in1=br_sb[:, 67:W1C].unsqueeze(1).broadcast_to([P, NT, QC]),
                op=mybir.AluOpType.add)
            b1_all = pp.tile([P, NT * 3], F32)
            nc.vector.tensor_tensor(
                out=b1_all[:].rearrange("p (t i) -> p t i", i=3),
                in0=c0_sb[:].unsqueeze(1).broadcast_to([P, NT, 3]),
                in1=v_all[:].rearrange("p (t i) -> p t i", i=3),
                op=mybir.AluOpType.subtract)
            nc.vector.tensor_tensor(
                out=base_sb[:], in0=b1_all[:], in1=pts_sb[:],
                op=mybir.AluOpType.add)

            table_pairs = table_full[:].rearrange("(a two) e -> a (two e)", two=2)
            chunks = []
            t0 = 0
            while t0 < n_tiles:
                chunks.append((t0, min(chunk_tiles, n_tiles - t0)))
                t0 += chunk_tiles

            # Descriptor generation is the bottleneck (~8ns/row-pair on the
            # Q7 SWDGE): PREPARE the first two chunks' gathers during
            # phase 1 + the collective, and fire them right after (Tile
            # defers the table RAW to the trigger).
            N_PREP = 0
            kxg_prep = []
            for ci in range(N_PREP):
                t0, nt = chunks[ci]
                nidx = nt * K * P
                kxg = kxgp.tile([P, nt * K * PAIR], F16, tag="kxg")
                gsem = nc.alloc_semaphore(f"gsem{ci}")
                nc.gpsimd.dma_gather(
                    kxg[:].rearrange("p (s e) -> p s e", e=PAIR),
                    table_pairs,
                    idx_sb[:, ds(t0 * K * 8, nt * K * 8)],
                    nidx, nidx, PAIR,
                    single_packet=False,
                    prepare_only=True, sem=gsem,
                )
                kxg_prep.append(kxg)

            # one DMA for the whole slice -> single wait for the collective
            nc.sync.dma_start(
                out=table_pad[:].rearrange("(t p) e -> p t e", p=P),
                in_=tblall_sb[:].rearrange("p (t e) -> p t e", e=ROW))

            # ---- all-gather the fp16 table ----
            nc.gpsimd.collective_compute(
                "AllGather",
                mybir.AluOpType.bypass,
                replica_groups=[list(range(N_CORES))],
                ins=[table_pad[ds(0, valid), :].opt()],
                outs=[table_full[:].opt()],
            )

            # ---- phase 2: pair-gather + attention ----
            for ci, (t0, nt) in enumerate(chunks):
                nidx = nt * K * P
                if ci < N_PREP:
                    kxg = kxg_prep[ci]
                else:
                    kxg = kxgp.tile([P, nt * K * PAIR], F16, tag="kxg")
                    nc.gpsimd.dma_gather(
                        kxg[:].rearrange("p (s e) -> p s e", e=PAIR),
                        table_pairs,
                        idx_sb[:, ds(t0 * K * 8, nt * K * 8)],
                        nidx, nidx, PAIR,
                        single_packet=False,
                        queue_num=ci % 4,
                    )
                kx4 = kxg[:].rearrange("p (t s e) -> p t s e",
                                       s=2 * K, e=ROW)
                qc_ap = (q_sb[:, ds(t0 * QC, nt * QC)]
                         .rearrange("p (t e) -> p t e", e=QC)
                         .unsqueeze(2).broadcast_to([P, nt, 2 * K, QC]))
                prod = wp.tile([P, nt * K * 2 * QC], F16, tag="prod")
                pr5 = prod[:].rearrange("p (t s e) -> p t s e",
                                        s=2 * K, e=QC)
                nc.vector.tensor_tensor(out=pr5, in0=kx4[:, :, :, 0:QC],
                                        in1=qc_ap, op=mybir.AluOpType.mult)
                s2a = wp.tile([P, nt * K * 2], F32, tag="s2a")
                nc.vector.tensor_reduce(out=s2a[:], in_=pr5,
                                        axis=mybir.AxisListType.X,
                                        op=mybir.AluOpType.add)
                sm2a = wp.tile([P, nt * K * 2], F32, tag="sm2a")
                nc.vector.tensor_tensor(out=sm2a[:], in0=s2a[:],
                                        in1=cm_sb[:, ds(t0 * K * 2, nt * K * 2)],
                                        op=mybir.AluOpType.mult)
                sma = wp.tile([P, nt * K], F32, tag="sma")
                nc.vector.tensor_reduce(
                    out=sma[:],
                    in_=sm2a[:].rearrange("p (tk h) -> p tk h", h=2),
                    axis=mybir.AxisListType.X, op=mybir.AluOpType.add)
                ea = wp.tile([P, nt * K], F16, tag="ea")
                nc.scalar.activation(out=ea[:], in_=sma[:],
                                     func=mybir.ActivationFunctionType.Exp)
                sea = wp.tile([P, nt], F32, tag="sea")
                nc.vector.tensor_reduce(
                    out=sea[:], in_=ea[:].rearrange("p (t k) -> p t k", k=K),
                    axis=mybir.AxisListType.X, op=mybir.AluOpType.add)
                ra = wp.tile([P, nt], F32, tag="ra")
                nc.vector.reciprocal(out=ra[:], in_=sea[:])
                esel = wp.tile([P, nt * K * 2], F16, tag="esel")
                nc.vector.tensor_tensor(
                    out=esel[:].rearrange("p (tk h) -> p tk h", h=2),
                    in0=pm_sb[:, ds(t0 * K * 2, nt * K * 2)]
                        .rearrange("p (tk h) -> p tk h", h=2),
                    in1=ea[:].unsqueeze(2).broadcast_to([P, nt * K, 2]),
                    op=mybir.AluOpType.mult)
                wpr = wp.tile([P, nt * K * 2 * 3], F32, tag="wpr")
                nc.vector.tensor_tensor(
                    out=wpr[:].rearrange("p (t s e) -> p t s e", s=2 * K, e=3),
                    in0=kx4[:, :, :, QC:QC + 3],
                    in1=esel[:].rearrange("p (t s) -> p t s", s=2 * K)
                        .unsqueeze(3).broadcast_to([P, nt, 2 * K, 3]),
                    op=mybir.AluOpType.mult)
                wsum = wp.tile([P, nt * 3], F32, tag="wsum")
                nc.vector.tensor_reduce(
                    out=wsum[:],
                    in_=wpr[:].rearrange("p (t s e) -> p t e s",
                                         s=2 * K, e=3),
                    axis=mybir.AxisListType.X, op=mybir.AluOpType.add)
                disp = wp.tile([P, nt * 3], F32, tag="disp")
                nc.vector.tensor_tensor(
                    out=disp[:].rearrange("p (t e) -> p t e", e=3),
                    in0=wsum[:].rearrange("p (t e) -> p t e", e=3),
                    in1=ra[:].unsqueeze(2).broadcast_to([P, nt, 3]),
                    op=mybir.AluOpType.mult)
                nc.vector.tensor_tensor(
                    out=out_sb[:, ds(t0 * 3, nt * 3)], in0=disp[:],
                    in1=base_sb[:, ds(t0 * 3, nt * 3)],
                    op=mybir.AluOpType.add)

            nc.sync.dma_start(out=out[:, :], in_=out_sb[:])

    nc.finalize()
    return nc


def prep_inputs(sampled_points, sampled_x, edge_index_filtered,
                W_concat, b_concat, W_out, b_out, W_q, b_q, W_k, b_k,
                n_total, shard, n_tiles):
    """Host-side layout prep + weight folding. Returns in_maps for 8 cores."""
    P = 128
    valid = n_total // N_CORES
    scale = np.sqrt(np.float32(H // 2), dtype=np.float32) + 1e-6

    Wc = W_concat.astype(np.float64)
    Wo = W_out.astype(np.float64)
    M1 = Wo @ Wc[:, :H]                                    # [3,128]
    M2 = Wo @ Wc[:, H:]                                    # [3,3]
    c0 = b_concat.astype(np.float64) @ Wo.T + b_out.astype(np.float64)

    w1 = np.zeros((P, W1C), np.float64)
    w1[:, 0:64] = W_k.astype(np.float64).T
    w1[:, 64:67] = M1.T
    w1[:, 67:W1C] = W_q.astype(np.float64).T / scale
    brow = np.zeros((1, W1C), np.float64)
    brow[0, 0:64] = b_k.astype(np.float64)
    brow[0, 67:W1C] = b_q.astype(np.float64) / scale

    w1 = w1.astype(np.float16)
    brow_rep = np.repeat(brow.astype(np.float16), P, 0)
    c0_rep = np.repeat(c0[None].astype(np.float32), P, 0)
    m2_rep = np.repeat(M2.reshape(1, 9).astype(np.float32), P, 0)

    dst = np.asarray(edge_index_filtered[1]).reshape(n_total, K)

    in_maps = []
    for r in range(N_CORES):
        rows = slice(r * valid, (r + 1) * valid)
        x_r = np.zeros((shard, H), np.float16)
        x_r[:valid] = sampled_x[rows].astype(np.float16)
        nb_r = np.zeros((shard, K), np.int64)
        nb_r[:valid] = dst[rows]
        pt_r = np.zeros((shard, 3), np.float32)
        pt_r[:valid] = sampled_points[rows].astype(np.float32)

        def swz(a, width):
            return (a.reshape(n_tiles, P, width).transpose(1, 0, 2)
                    .reshape(P, n_tiles * width).copy())

        # gather indices: position (slot = t*K+k, p) -> idx = nbr//2, stored
        # int16 wrapped-16: [16, pos//16] replicated to all 8 partition groups
        nbs = nb_r.reshape(n_tiles, P, K)
        npos = n_tiles * K * P
        stream = np.empty(npos, np.int64)
        pos = np.arange(npos)
        slot, p = pos // P, pos % P
        t_, k_ = slot // K, slot % K
        stream = nbs[t_, p, k_]
        idxw = (stream // 2).astype(np.int16).reshape(-1, 16).T  # [16, npos/16]
        idx_rep = np.tile(idxw, (8, 1))                          # [128, npos/16]

        par = (stream % 2).astype(np.float32)                    # h=1 half
        nz = (stream != 0).astype(np.float32)
        # masks laid out [p, (t k h)]
        pmask = np.zeros((P, n_tiles * K * 2), np.float32)
        pmask[p, (t_ * K + k_) * 2 + 0] = 1.0 - par
        pmask[p, (t_ * K + k_) * 2 + 1] = par
        cmask = pmask.copy()
        cmask[p, (t_ * K + k_) * 2 + 0] *= nz
        cmask[p, (t_ * K + k_) * 2 + 1] *= nz

        in_maps.append({
            "ax": np.ascontiguousarray(x_r.T),
            "pts": swz(pt_r, 3),
            "idx16": np.ascontiguousarray(idx_rep),
            "cmask": cmask.astype(np.float32),
            "pmask": pmask.astype(np.float16),
            "w1": w1,
            "brow": brow_rep,
            "c0r": c0_rep,
            "m2r": m2_rep,
        })
    return in_maps


def assemble_output(results, n_total, n_tiles):
    P = 128
    valid = n_total // N_CORES
    outs = []
    for r in range(N_CORES):
        o = results[r]["out"]
        o = (o.reshape(P, n_tiles, 3).transpose(1, 0, 2)
             .reshape(n_tiles * P, 3)[:valid])
        outs.append(o)
    return np.concatenate(outs, axis=0).astype(np.float32)


_CACHED = {}


def _get_program(n_total, shard, n_tiles):
    key = (n_total, shard, n_tiles)
    if key not in _CACHED:
        _CACHED[key] = build_program(n_total, shard, n_tiles)
    return _CACHED[key]


def kernel(sampled_points, sampled_x, edge_index_filtered,
           W_concat, b_concat, W_out, b_out, W_q, b_q, W_k, b_k):
    n_total = 60000
    n_tiles = 59
    shard = n_tiles * 128
    nc = _get_program(n_total, shard, n_tiles)
    in_maps = prep_inputs(
        np.asarray(sampled_points), np.asarray(sampled_x),
        np.asarray(edge_index_filtered),
        np.asarray(W_concat), np.asarray(b_concat),
        np.asarray(W_out), np.asarray(b_out),
        np.asarray(W_q), np.asarray(b_q),
        np.asarray(W_k), np.asarray(b_k),
        n_total, shard, n_tiles)
    res = run_bass_kernel_spmd(nc, in_maps, list(range(N_CORES)))
    return assemble_output(res.results, n_total, n_tiles)



# revision 14
# speedup vs baseline: 3.2601x; 3.2601x over previous
"""Trainium2 Bass kernel for MeshGNN message passing (8 NeuronCores, SPMD).

Math reformulation (exact): since softmax weights sum to 1 and the output MLP is
linear, fold W_concat/W_out into per-node quantities:
    M1 = W_out @ W_concat[:, :128]   [3,128]
    M2 = W_out @ W_concat[:, 128:]   [3,3]
    c0 = b_concat @ W_out.T + b_out  [3]
    kx[j] = x[j] @ W_k.T + b_k                  (64,)   -> table
    w[j]  = x[j] @ M1.T + p[j] @ M2.T           (3,)    -> table
    q[n]  = (x[n] @ W_q.T + b_q) / scale        (64,)
    scores[n,k] = q[n] . kx[nbr]
    e = exp(scores * (nbr != 0))                         (scores bounded ~±3)
    out[n] = p[n] + (-v[n] + c0) + sum_k e_k * w[nbr] / sum_k e_k,  v = p @ M2.T

Implementation: per-node table rows of 128 fp16 (=256B): [kx(64)|w(3)|pad61].
Rows are fetched with dma_gather in PAIRS (512B, idx = nbr//2 fits int16),
and the correct half is selected arithmetically via host-prepared parity
masks folded into the score/softmax math (multi-row indirect_dma_start is
broken on HW; dma_gather is the production batched-gather path).
Phase 1 computes the fp16 table + q + base per 128-node tile with one matmul;
an AllGather shares the table; phase 2 gathers pairs chunk-wise and runs the
attention on DVE/ACT.
"""

import sys

import numpy as np

sys.path.insert(0, "/opt/trn_rl_repo")

import concourse.bass as bass
import concourse.mybir as mybir
import concourse.tile as tile
from concourse import bacc
from concourse.bass import ds, ts
from concourse.bass_utils import run_bass_kernel_spmd

N_CORES = 8
H = 128
K = 15
DT = mybir.dt
F16 = DT.float16
F32 = DT.float32
I16 = DT.int16

ROW = 128                 # fp16 elems per table row (256B)
PAIR = 2 * ROW            # gather element: two rows (512B)
QC = 64                   # q/k dim
W1C = 131                 # matmul cols: kx(64)|w(3)|q(64)


def build_program(n_total, shard, n_tiles, chunk_tiles=3):
    valid = n_total // N_CORES
    P = 128
    nc = bacc.Bacc(None, debug=False, num_swdge_queues=4)

    ax = nc.declare_dram_parameter("ax", [P, shard], F16, isOutput=False)    # x.T
    pts = nc.declare_dram_parameter("pts", [P, n_tiles * 3], F32, isOutput=False)
    idx16 = nc.declare_dram_parameter("idx16", [P, n_tiles * K * 8], I16,
                                      isOutput=False)
    cmask = nc.declare_dram_parameter("cmask", [P, n_tiles * K * 2], F16,
                                      isOutput=False)
    pmask = nc.declare_dram_parameter("pmask", [P, n_tiles * K * 2], F16,
                                      isOutput=False)
    w1 = nc.declare_dram_parameter("w1", [P, W1C], F16, isOutput=False)
    brow = nc.declare_dram_parameter("brow", [P, W1C], F16, isOutput=False)
    c0r = nc.declare_dram_parameter("c0r", [P, 3], F32, isOutput=False)
    m2r = nc.declare_dram_parameter("m2r", [P, 9], F32, isOutput=False)
    out = nc.declare_dram_parameter("out", [P, n_tiles * 3], F32, isOutput=True)

    with tile.TileContext(nc) as tc:
        with (
            tc.tile_pool(name="persist", bufs=1) as pp,
            tc.tile_pool(name="dram", bufs=1, space="DRAM") as dp,
            tc.tile_pool(name="psum", bufs=4, space="PSUM") as psp,
            tc.tile_pool(name="kxgp", bufs=5) as kxgp,
            tc.tile_pool(name="work", bufs=2) as wp,
        ):
            # ---- persistent SBUF ----
            xT = pp.tile([P, shard], F16)
            pts_sb = pp.tile([P, n_tiles * 3], F32)
            idx_sb = pp.tile([P, n_tiles * K * 8], I16)
            cm_sb = pp.tile([P, n_tiles * K * 2], F16)
            pm_sb = pp.tile([P, n_tiles * K * 2], F16)
            w1_sb = pp.tile([P, W1C], F16)
            br_sb = pp.tile([P, W1C], F16)
            c0_sb = pp.tile([P, 3], F32)
            m2_sb = pp.tile([P, 9], F32)
            q_sb = pp.tile([P, n_tiles * QC], F16)
            base_sb = pp.tile([P, n_tiles * 3], F32)
            out_sb = pp.tile([P, n_tiles * 3], F32)
            stage_sb = kxgp.tile([P, chunk_tiles * K * PAIR], F16,
                                 tag="kxg")
            tblall_sb = kxgp.tile([P, chunk_tiles * K * PAIR], F16,
                                  tag="kxg")

            table_pad = dp.tile([shard, ROW], F16, space="DRAM")
            table_full = dp.tile([n_total, ROW], F16, space="DRAM",
                                 addr_space="Shared")

            nc.sync.dma_start(out=xT[:], in_=ax[:, :])
            nc.sync.dma_start(out=pts_sb[:], in_=pts[:, :])
            nc.sync.dma_start(out=idx_sb[:], in_=idx16[:, :])
            nc.sync.dma_start(out=cm_sb[:], in_=cmask[:, :])
            nc.sync.dma_start(out=pm_sb[:], in_=pmask[:, :])
            nc.sync.dma_start(out=w1_sb[:], in_=w1[:, :])
            nc.sync.dma_start(out=br_sb[:], in_=brow[:, :])
            nc.sync.dma_start(out=c0_sb[:], in_=c0r[:, :])
            nc.sync.dma_start(out=m2_sb[:], in_=m2r[:, :])

            nc.vector.memset(tblall_sb[:], 0)

            # ---- phase 1: matmuls -> ACT-staged copies -> batched DVE ----
            # Matmuls for 3 tiles share one PSUM tile (3*131 fp32 = 1572B
            # fits a 2KB bank) so the ACT copy's ~700ns fixed cost is paid
            # once per 3 tiles instead of per tile.
            NT = n_tiles
            t0 = 0
            while t0 < n_tiles:
                g = min(3, n_tiles - t0)
                ps = psp.tile([P, g * W1C], F32, space="PSUM", tag="ps")
                for j in range(g):
                    nc.tensor.matmul(out=ps[:, ts(j, W1C)],
                                     lhsT=xT[:, ts(t0 + j, P)], rhs=w1_sb[:],
                                     start=True, stop=True)
                nc.scalar.copy(out=stage_sb[:, ds(t0 * W1C, g * W1C)],
                               in_=ps[:])
                t0 += g

            st3 = stage_sb[:, 0:NT * W1C].rearrange("p (t c) -> p t c", c=W1C)
            # v = p @ M2.T for all tiles: [P, NT, 3]
            vp_all = pp.tile([P, NT * 9], F32)
            for j in range(3):
                nc.vector.tensor_tensor(
                    out=vp_all[:].rearrange("p (t j i) -> p t j i", j=3, i=3)
                        [:, :, j, :],
                    in0=pts_sb[:].rearrange("p (t i) -> p t i", i=3),
                    in1=m2_sb[:, ds(3 * j, 3)].unsqueeze(1)
                        .broadcast_to([P, NT, 3]),
                    op=mybir.AluOpType.mult)
            v_all = pp.tile([P, NT * 3], F32)
            nc.vector.tensor_reduce(
                out=v_all[:],
                in_=vp_all[:].rearrange("p (t j i) -> p (t j) i", j=3, i=3),
                axis=mybir.AxisListType.X, op=mybir.AluOpType.add)
            v16 = pp.tile([P, NT * 3], F16)
            nc.vector.tensor_copy(out=v16[:], in_=v_all[:])
            tbl3 = tblall_sb[:, 0:NT * ROW].rearrange("p (t e) -> p t e", e=ROW)
            nc.vector.tensor_tensor(
                out=tbl3[:, :, 0:67], in0=st3[:, :, 0:67],
                in1=br_sb[:, 0:67].unsqueeze(1).broadcast_to([P, NT, 67]),
                op=mybir.AluOpType.add)
            nc.vector.tensor_tensor(
                out=tbl3[:, :, 64:67], in0=tbl3[:, :, 64:67],
                in1=v16[:].rearrange("p (t i) -> p t i", i=3),
                op=mybir.AluOpType.add)
            nc.vector.tensor_tensor(
                out=q_sb[:].rearrange("p (t e) -> p t e", e=QC),
                in0=st3[:, :, 67:W1C],
                in1=br_sb[:, 67:W1C].unsqueeze(1).broadcast_to([P, NT, QC]),
                op=mybir.AluOpType.add)
            b1_all = pp.tile([P, NT * 3], F32)
            nc.vector.tensor_tensor(
                out=b1_all[:].rearrange("p (t i) -> p t i", i=3),
                in0=c0_sb[:].unsqueeze(1).broadcast_to([P, NT, 3]),
                in1=v_all[:].rearrange("p (t i) -> p t i", i=3),
                op=mybir.AluOpType.subtract)
            nc.vector.tensor_tensor(
                out=base_sb[:], in0=b1_all[:], in1=pts_sb[:],
                op=mybir.AluOpType.add)

            table_pairs = table_full[:].rearrange("(a two) e -> a (two e)", two=2)
            chunks = []
            t0 = 0
            while t0 < n_tiles:
                chunks.append((t0, min(chunk_tiles, n_tiles - t0)))
                t0 += chunk_tiles

            # one DMA for the whole slice -> single wait for the collective
            nc.sync.dma_start(
                out=table_pad[:].rearrange("(t p) e -> p t e", p=P),
                in_=tblall_sb[:, 0:NT * ROW].rearrange("p (t e) -> p t e", e=ROW))

            # ---- all-gather the fp16 table ----
            nc.gpsimd.collective_compute(
                "AllGather",
                mybir.AluOpType.bypass,
                replica_groups=[list(range(N_CORES))],
                ins=[table_pad[ds(0, valid), :].opt()],
                outs=[table_full[:].opt()],
            )

            # ---- phase 2: pair-gather + attention ----
            def emit_gather(ci):
                t0, nt = chunks[ci]
                nidx = nt * K * P
                kxg = kxgp.tile([P, nt * K * PAIR], F16, tag="kxg")
                nc.gpsimd.dma_gather(
                    kxg[:].rearrange("p (s e) -> p s e", e=PAIR),
                    table_pairs,
                    idx_sb[:, ds(t0 * K * 8, nt * K * 8)],
                    nidx, nidx, PAIR,
                    single_packet=False,
                    queue_num=ci % 4,
                )
                return kxg


            def emit_compute(ci, kxg):
                t0, nt = chunks[ci]
                kx4 = kxg[:].rearrange("p (t s e) -> p t s e",
                                       s=2 * K, e=ROW)
                qc_ap = (q_sb[:, ds(t0 * QC, nt * QC)]
                         .rearrange("p (t e) -> p t e", e=QC)
                         .unsqueeze(2).broadcast_to([P, nt, 2 * K, QC]))
                prod = wp.tile([P, nt * K * 2 * QC], F16, tag="prod")
                pr5 = prod[:].rearrange("p (t s e) -> p t s e",
                                        s=2 * K, e=QC)
                nc.vector.tensor_tensor(out=pr5, in0=kx4[:, :, :, 0:QC],
                                        in1=qc_ap, op=mybir.AluOpType.mult)
                s2a = wp.tile([P, nt * K * 2], F16, tag="s2a")
                nc.vector.tensor_reduce(out=s2a[:], in_=pr5,
                                        axis=mybir.AxisListType.X,
                                        op=mybir.AluOpType.add)
                sm2a = wp.tile([P, nt * K * 2], F16, tag="sm2a")
                nc.vector.tensor_tensor(out=sm2a[:], in0=s2a[:],
                                        in1=cm_sb[:, ds(t0 * K * 2, nt * K * 2)],
                                        op=mybir.AluOpType.mult)
                sma = wp.tile([P, nt * K], F16, tag="sma")
                nc.vector.tensor_reduce(
                    out=sma[:],
                    in_=sm2a[:].rearrange("p (tk h) -> p tk h", h=2),
                    axis=mybir.AxisListType.X, op=mybir.AluOpType.add)
                ea = wp.tile([P, nt * K], F16, tag="ea")
                nc.scalar.activation(out=ea[:], in_=sma[:],
                                     func=mybir.ActivationFunctionType.Exp)
                sea = wp.tile([P, nt], F16, tag="sea")
                nc.vector.tensor_reduce(
                    out=sea[:], in_=ea[:].rearrange("p (t k) -> p t k", k=K),
                    axis=mybir.AxisListType.X, op=mybir.AluOpType.add)
                ra = wp.tile([P, nt], F16, tag="ra")
                nc.vector.reciprocal(out=ra[:], in_=sea[:])
                esel = wp.tile([P, nt * K * 2], F16, tag="esel")
                nc.vector.tensor_tensor(
                    out=esel[:].rearrange("p (tk h) -> p tk h", h=2),
                    in0=pm_sb[:, ds(t0 * K * 2, nt * K * 2)]
                        .rearrange("p (tk h) -> p tk h", h=2),
                    in1=ea[:].unsqueeze(2).broadcast_to([P, nt * K, 2]),
                    op=mybir.AluOpType.mult)
                wpr = wp.tile([P, nt * K * 2 * 3], F16, tag="wpr")
                nc.vector.tensor_tensor(
                    out=wpr[:].rearrange("p (t s e) -> p t s e", s=2 * K, e=3),
                    in0=kx4[:, :, :, QC:QC + 3],
                    in1=esel[:].rearrange("p (t s) -> p t s", s=2 * K)
                        .unsqueeze(3).broadcast_to([P, nt, 2 * K, 3]),
                    op=mybir.AluOpType.mult)
                wsum = wp.tile([P, nt * 3], F16, tag="wsum")
                nc.vector.tensor_reduce(
                    out=wsum[:],
                    in_=wpr[:].rearrange("p (t s e) -> p t e s",
                                         s=2 * K, e=3),
                    axis=mybir.AxisListType.X, op=mybir.AluOpType.add)
                disp = wp.tile([P, nt * 3], F32, tag="disp")
                nc.vector.tensor_tensor(
                    out=disp[:].rearrange("p (t e) -> p t e", e=3),
                    in0=wsum[:].rearrange("p (t e) -> p t e", e=3),
                    in1=ra[:].unsqueeze(2).broadcast_to([P, nt, 3]),
                    op=mybir.AluOpType.mult)
                nc.vector.tensor_tensor(
                    out=out_sb[:, ds(t0 * 3, nt * 3)], in0=disp[:],
                    in1=base_sb[:, ds(t0 * 3, nt * 3)],
                    op=mybir.AluOpType.add)

            with nc.allow_low_precision(
                    reason="fp16 attention intermediates; rel-err budget 2e-2"):
                for ci in range(len(chunks)):
                    emit_compute(ci, emit_gather(ci))

            nc.sync.dma_start(out=out[:, :], in_=out_sb[:])

    nc.finalize()
    return nc


def prep_inputs(sampled_points, sampled_x, edge_index_filtered,
                W_concat, b_concat, W_out, b_out, W_q, b_q, W_k, b_k,
                n_total, shard, n_tiles):
    """Host-side layout prep + weight folding. Returns in_maps for 8 cores."""
    P = 128
    valid = n_total // N_CORES
    scale = np.sqrt(np.float32(H // 2), dtype=np.float32) + 1e-6

    Wc = W_concat.astype(np.float64)
    Wo = W_out.astype(np.float64)
    M1 = Wo @ Wc[:, :H]                                    # [3,128]
    M2 = Wo @ Wc[:, H:]                                    # [3,3]
    c0 = b_concat.astype(np.float64) @ Wo.T + b_out.astype(np.float64)

    w1 = np.zeros((P, W1C), np.float64)
    w1[:, 0:64] = W_k.astype(np.float64).T
    w1[:, 64:67] = M1.T
    w1[:, 67:W1C] = W_q.astype(np.float64).T / scale
    brow = np.zeros((1, W1C), np.float64)
    brow[0, 0:64] = b_k.astype(np.float64)
    brow[0, 67:W1C] = b_q.astype(np.float64) / scale

    w1 = w1.astype(np.float16)
    brow_rep = np.repeat(brow.astype(np.float16), P, 0)
    c0_rep = np.repeat(c0[None].astype(np.float32), P, 0)
    m2_rep = np.repeat(M2.reshape(1, 9).astype(np.float32), P, 0)

    dst = np.asarray(edge_index_filtered[1]).reshape(n_total, K)

    in_maps = []
    for r in range(N_CORES):
        rows = slice(r * valid, (r + 1) * valid)
        x_r = np.zeros((shard, H), np.float16)
        x_r[:valid] = sampled_x[rows].astype(np.float16)
        nb_r = np.zeros((shard, K), np.int64)
        nb_r[:valid] = dst[rows]
        pt_r = np.zeros((shard, 3), np.float32)
        pt_r[:valid] = sampled_points[rows].astype(np.float32)

        def swz(a, width):
            return (a.reshape(n_tiles, P, width).transpose(1, 0, 2)
                    .reshape(P, n_tiles * width).copy())

        # gather indices: position (slot = t*K+k, p) -> idx = nbr//2, stored
        # int16 wrapped-16: [16, pos//16] replicated to all 8 partition groups
        nbs = nb_r.reshape(n_tiles, P, K)
        npos = n_tiles * K * P
        stream = np.empty(npos, np.int64)
        pos = np.arange(npos)
        slot, p = pos // P, pos % P
        t_, k_ = slot // K, slot % K
        stream = nbs[t_, p, k_]
        idxw = (stream // 2).astype(np.int16).reshape(-1, 16).T  # [16, npos/16]
        idx_rep = np.tile(idxw, (8, 1))                          # [128, npos/16]

        par = (stream % 2).astype(np.float32)                    # h=1 half
        nz = (stream != 0).astype(np.float32)
        # masks laid out [p, (t k h)]
        pmask = np.zeros((P, n_tiles * K * 2), np.float32)
        pmask[p, (t_ * K + k_) * 2 + 0] = 1.0 - par
        pmask[p, (t_ * K + k_) * 2 + 1] = par
        cmask = pmask.copy()
        cmask[p, (t_ * K + k_) * 2 + 0] *= nz
        cmask[p, (t_ * K + k_) * 2 + 1] *= nz

        in_maps.append({
            "ax": np.ascontiguousarray(x_r.T),
            "pts": swz(pt_r, 3),
            "idx16": np.ascontiguousarray(idx_rep),
            "cmask": cmask.astype(np.float16),
            "pmask": pmask.astype(np.float16),
            "w1": w1,
            "brow": brow_rep,
            "c0r": c0_rep,
            "m2r": m2_rep,
        })
    return in_maps


def assemble_output(results, n_total, n_tiles):
    P = 128
    valid = n_total // N_CORES
    outs = []
    for r in range(N_CORES):
        o = results[r]["out"]
        o = (o.reshape(P, n_tiles, 3).transpose(1, 0, 2)
             .reshape(n_tiles * P, 3)[:valid])
        outs.append(o)
    return np.concatenate(outs, axis=0).astype(np.float32)


_CACHED = {}


def _get_program(n_total, shard, n_tiles):
    key = (n_total, shard, n_tiles)
    if key not in _CACHED:
        _CACHED[key] = build_program(n_total, shard, n_tiles)
    return _CACHED[key]


def kernel(sampled_points, sampled_x, edge_index_filtered,
           W_concat, b_concat, W_out, b_out, W_q, b_q, W_k, b_k):
    n_total = 60000
    n_tiles = 59
    shard = n_tiles * 128
    nc = _get_program(n_total, shard, n_tiles)
    in_maps = prep_inputs(
        np.asarray(sampled_points), np.asarray(sampled_x),
        np.asarray(edge_index_filtered),
        np.asarray(W_concat), np.asarray(b_concat),
        np.asarray(W_out), np.asarray(b_out),
        np.asarray(W_q), np.asarray(b_q),
        np.asarray(W_k), np.asarray(b_k),
        n_total, shard, n_tiles)
    res = run_bass_kernel_spmd(nc, in_maps, list(range(N_CORES)))
    return assemble_output(res.results, n_total, n_tiles)



# revision 15
# speedup vs baseline: 4.4501x; 1.3650x over previous
"""Trainium2 Bass kernel for MeshGNN message passing (8 NeuronCores, SPMD).

Math reformulation (exact): since softmax weights sum to 1 and the output MLP is
linear, fold W_concat/W_out into per-node quantities:
    M1 = W_out @ W_concat[:, :128]   [3,128]
    M2 = W_out @ W_concat[:, 128:]   [3,3]
    c0 = b_concat @ W_out.T + b_out  [3]
    kx[j] = x[j] @ W_k.T + b_k                  (64,)   -> table
    w[j]  = x[j] @ M1.T + p[j] @ M2.T           (3,)    -> table
    q[n]  = (x[n] @ W_q.T + b_q) / scale        (64,)
    scores[n,k] = q[n] . kx[nbr]
    e = exp(scores * (nbr != 0))                         (scores bounded ~±3)
    out[n] = p[n] + (-v[n] + c0) + sum_k e_k * w[nbr] / sum_k e_k,  v = p @ M2.T

Implementation: per-node table rows of 128 fp16 (=256B): [kx(64)|w(3)|pad61].
Rows are fetched with dma_gather in PAIRS (512B, idx = nbr//2 fits int16),
and the correct half is selected arithmetically via host-prepared parity
masks folded into the score/softmax math (multi-row indirect_dma_start is
broken on HW; dma_gather is the production batched-gather path).
Phase 1 computes the fp16 table + q + base per 128-node tile with one matmul;
an AllGather shares the table; phase 2 gathers pairs chunk-wise and runs the
attention on DVE/ACT.
"""

import sys

import numpy as np

sys.path.insert(0, "/opt/trn_rl_repo")

import concourse.bass as bass
import concourse.mybir as mybir
import concourse.tile as tile
from concourse import bacc
from concourse.bass import ds, ts
from concourse.bass_utils import run_bass_kernel_spmd

N_CORES = 8
H = 128
K = 15
DT = mybir.dt
F16 = DT.float16
F32 = DT.float32
I16 = DT.int16

ROW = 128                 # fp16 elems per table row (256B)
PAIR = 2 * ROW            # gather element: two rows (512B)
QC = 64                   # q/k dim
W1C = 131                 # matmul cols: kx(64)|w(3)|q(64)


def build_program(n_total, shard, n_tiles, chunk_tiles=2):
    valid = n_total // N_CORES
    P = 128
    nc = bacc.Bacc(None, debug=False, num_swdge_queues=4)

    ax = nc.declare_dram_parameter("ax", [P, shard], F16, isOutput=False)    # x.T
    pts = nc.declare_dram_parameter("pts", [P, n_tiles * 3], F32, isOutput=False)
    idx16 = nc.declare_dram_parameter("idx16", [P, n_tiles * K * 8], I16,
                                      isOutput=False)
    cmask = nc.declare_dram_parameter("cmask", [P, n_tiles * K * 2], F16,
                                      isOutput=False)
    pmask = nc.declare_dram_parameter("pmask", [P, n_tiles * K * 2], F16,
                                      isOutput=False)
    w1 = nc.declare_dram_parameter("w1", [P, W1C], F16, isOutput=False)
    brow = nc.declare_dram_parameter("brow", [P, W1C], F16, isOutput=False)
    c0r = nc.declare_dram_parameter("c0r", [P, 3], F32, isOutput=False)
    m2r = nc.declare_dram_parameter("m2r", [P, 9], F32, isOutput=False)
    out = nc.declare_dram_parameter("out", [P, n_tiles * 3], F32, isOutput=True)

    with tile.TileContext(nc) as tc:
        with (
            tc.tile_pool(name="persist", bufs=1) as pp,
            tc.tile_pool(name="dram", bufs=1, space="DRAM") as dp,
            tc.tile_pool(name="psum", bufs=4, space="PSUM") as psp,
            tc.tile_pool(name="kxgp", bufs=7) as kxgp,
            tc.tile_pool(name="work", bufs=2) as wp,
        ):
            # ---- persistent SBUF ----
            xT = pp.tile([P, shard], F16)
            pts_sb = pp.tile([P, n_tiles * 3], F32)
            idx_sb = pp.tile([P, n_tiles * K * 8], I16)
            cm_sb = pp.tile([P, n_tiles * K * 2], F16)
            pm_sb = pp.tile([P, n_tiles * K * 2], F16)
            w1_sb = pp.tile([P, W1C], F16)
            br_sb = pp.tile([P, W1C], F16)
            c0_sb = pp.tile([P, 3], F32)
            m2_sb = pp.tile([P, 9], F32)
            q_sb = pp.tile([P, n_tiles * QC], F16)
            base_sb = pp.tile([P, n_tiles * 3], F32)
            out_sb = pp.tile([P, n_tiles * 3], F32)
            stage_sb = kxgp.tile([P, n_tiles * W1C], F16,
                                 tag="kxg")
            tblall_sb = kxgp.tile([P, n_tiles * ROW], F16,
                                  tag="kxg")

            table_pad = dp.tile([shard, ROW], F16, space="DRAM")
            table_full = dp.tile([n_total, ROW], F16, space="DRAM",
                                 addr_space="Shared")

            nc.sync.dma_start(out=xT[:], in_=ax[:, :])
            nc.sync.dma_start(out=pts_sb[:], in_=pts[:, :])
            nc.sync.dma_start(out=idx_sb[:], in_=idx16[:, :])
            nc.sync.dma_start(out=cm_sb[:], in_=cmask[:, :])
            nc.sync.dma_start(out=pm_sb[:], in_=pmask[:, :])
            nc.sync.dma_start(out=w1_sb[:], in_=w1[:, :])
            nc.sync.dma_start(out=br_sb[:], in_=brow[:, :])
            nc.sync.dma_start(out=c0_sb[:], in_=c0r[:, :])
            nc.sync.dma_start(out=m2_sb[:], in_=m2r[:, :])

            nc.vector.memset(tblall_sb[:], 0)

            # ---- phase 1: matmuls -> ACT-staged copies -> batched DVE ----
            # Matmuls for 3 tiles share one PSUM tile (3*131 fp32 = 1572B
            # fits a 2KB bank) so the ACT copy's ~700ns fixed cost is paid
            # once per 3 tiles instead of per tile.
            NT = n_tiles
            t0 = 0
            while t0 < n_tiles:
                g = min(3, n_tiles - t0)
                ps = psp.tile([P, g * W1C], F32, space="PSUM", tag="ps")
                for j in range(g):
                    nc.tensor.matmul(out=ps[:, ts(j, W1C)],
                                     lhsT=xT[:, ts(t0 + j, P)], rhs=w1_sb[:],
                                     start=True, stop=True)
                nc.scalar.copy(out=stage_sb[:, ds(t0 * W1C, g * W1C)],
                               in_=ps[:])
                t0 += g

            st3 = stage_sb[:, 0:NT * W1C].rearrange("p (t c) -> p t c", c=W1C)
            # v = p @ M2.T for all tiles: [P, NT, 3]
            vp_all = pp.tile([P, NT * 9], F32)
            for j in range(3):
                nc.vector.tensor_tensor(
                    out=vp_all[:].rearrange("p (t j i) -> p t j i", j=3, i=3)
                        [:, :, j, :],
                    in0=pts_sb[:].rearrange("p (t i) -> p t i", i=3),
                    in1=m2_sb[:, ds(3 * j, 3)].unsqueeze(1)
                        .broadcast_to([P, NT, 3]),
                    op=mybir.AluOpType.mult)
            v_all = pp.tile([P, NT * 3], F32)
            nc.vector.tensor_reduce(
                out=v_all[:],
                in_=vp_all[:].rearrange("p (t j i) -> p (t j) i", j=3, i=3),
                axis=mybir.AxisListType.X, op=mybir.AluOpType.add)
            v16 = pp.tile([P, NT * 3], F16)
            nc.vector.tensor_copy(out=v16[:], in_=v_all[:])
            tbl3 = tblall_sb[:, 0:NT * ROW].rearrange("p (t e) -> p t e", e=ROW)
            nc.vector.tensor_tensor(
                out=tbl3[:, :, 0:67], in0=st3[:, :, 0:67],
                in1=br_sb[:, 0:67].unsqueeze(1).broadcast_to([P, NT, 67]),
                op=mybir.AluOpType.add)
            nc.vector.tensor_tensor(
                out=tbl3[:, :, 64:67], in0=tbl3[:, :, 64:67],
                in1=v16[:].rearrange("p (t i) -> p t i", i=3),
                op=mybir.AluOpType.add)
            nc.vector.tensor_tensor(
                out=q_sb[:].rearrange("p (t e) -> p t e", e=QC),
                in0=st3[:, :, 67:W1C],
                in1=br_sb[:, 67:W1C].unsqueeze(1).broadcast_to([P, NT, QC]),
                op=mybir.AluOpType.add)
            b1_all = pp.tile([P, NT * 3], F32)
            nc.vector.tensor_tensor(
                out=b1_all[:].rearrange("p (t i) -> p t i", i=3),
                in0=c0_sb[:].unsqueeze(1).broadcast_to([P, NT, 3]),
                in1=v_all[:].rearrange("p (t i) -> p t i", i=3),
                op=mybir.AluOpType.subtract)
            nc.vector.tensor_tensor(
                out=base_sb[:], in0=b1_all[:], in1=pts_sb[:],
                op=mybir.AluOpType.add)

            table_pairs = table_full[:].rearrange("(a two) e -> a (two e)", two=2)
            chunks = []
            t0 = 0
            while t0 < n_tiles:
                chunks.append((t0, min(chunk_tiles, n_tiles - t0)))
                t0 += chunk_tiles

            # one DMA for the whole slice -> single wait for the collective
            nc.sync.dma_start(
                out=table_pad[:].rearrange("(t p) e -> p t e", p=P),
                in_=tblall_sb[:, 0:NT * ROW].rearrange("p (t e) -> p t e", e=ROW))

            # ---- all-gather the fp16 table ----
            nc.gpsimd.collective_compute(
                "AllGather",
                mybir.AluOpType.bypass,
                replica_groups=[list(range(N_CORES))],
                ins=[table_pad[ds(0, valid), :].opt()],
                outs=[table_full[:].opt()],
            )

            # ---- phase 2: pair-gather + attention ----
            def emit_gather(ci):
                t0, nt = chunks[ci]
                nidx = nt * K * P
                kxg = kxgp.tile([P, nt * K * PAIR], F16, tag="kxg")
                nc.gpsimd.dma_gather(
                    kxg[:].rearrange("p (s e) -> p s e", e=PAIR),
                    table_pairs,
                    idx_sb[:, ds(t0 * K * 8, nt * K * 8)],
                    nidx, nidx, PAIR,
                    single_packet=False,
                    queue_num=ci % 4,
                )
                return kxg


            def emit_compute(ci, kxg):
                t0, nt = chunks[ci]
                kx4 = kxg[:].rearrange("p (t s e) -> p t s e",
                                       s=2 * K, e=ROW)
                qc_ap = (q_sb[:, ds(t0 * QC, nt * QC)]
                         .rearrange("p (t e) -> p t e", e=QC)
                         .unsqueeze(2).broadcast_to([P, nt, 2 * K, QC]))
                prod = wp.tile([P, nt * K * 2 * QC], F16, tag="prod")
                pr5 = prod[:].rearrange("p (t s e) -> p t s e",
                                        s=2 * K, e=QC)
                nc.vector.tensor_tensor(out=pr5, in0=kx4[:, :, :, 0:QC],
                                        in1=qc_ap, op=mybir.AluOpType.mult)
                s2a = wp.tile([P, nt * K * 2], F16, tag="s2a")
                nc.vector.tensor_reduce(out=s2a[:], in_=pr5,
                                        axis=mybir.AxisListType.X,
                                        op=mybir.AluOpType.add)
                sm2a = wp.tile([P, nt * K * 2], F16, tag="sm2a")
                nc.vector.tensor_tensor(out=sm2a[:], in0=s2a[:],
                                        in1=cm_sb[:, ds(t0 * K * 2, nt * K * 2)],
                                        op=mybir.AluOpType.mult)
                sma = wp.tile([P, nt * K], F16, tag="sma")
                nc.vector.tensor_reduce(
                    out=sma[:],
                    in_=sm2a[:].rearrange("p (tk h) -> p tk h", h=2),
                    axis=mybir.AxisListType.X, op=mybir.AluOpType.add)
                ea = wp.tile([P, nt * K], F16, tag="ea")
                nc.scalar.activation(out=ea[:], in_=sma[:],
                                     func=mybir.ActivationFunctionType.Exp)
                sea = wp.tile([P, nt], F16, tag="sea")
                nc.vector.tensor_reduce(
                    out=sea[:], in_=ea[:].rearrange("p (t k) -> p t k", k=K),
                    axis=mybir.AxisListType.X, op=mybir.AluOpType.add)
                ra = wp.tile([P, nt], F16, tag="ra")
                nc.vector.reciprocal(out=ra[:], in_=sea[:])
                esel = wp.tile([P, nt * K * 2], F16, tag="esel")
                nc.vector.tensor_tensor(
                    out=esel[:].rearrange("p (tk h) -> p tk h", h=2),
                    in0=pm_sb[:, ds(t0 * K * 2, nt * K * 2)]
                        .rearrange("p (tk h) -> p tk h", h=2),
                    in1=ea[:].unsqueeze(2).broadcast_to([P, nt * K, 2]),
                    op=mybir.AluOpType.mult)
                wpr = wp.tile([P, nt * K * 2 * 3], F16, tag="wpr")
                nc.vector.tensor_tensor(
                    out=wpr[:].rearrange("p (t s e) -> p t s e", s=2 * K, e=3),
                    in0=kx4[:, :, :, QC:QC + 3],
                    in1=esel[:].rearrange("p (t s) -> p t s", s=2 * K)
                        .unsqueeze(3).broadcast_to([P, nt, 2 * K, 3]),
                    op=mybir.AluOpType.mult)
                wsum = wp.tile([P, nt * 3], F16, tag="wsum")
                nc.vector.tensor_reduce(
                    out=wsum[:],
                    in_=wpr[:].rearrange("p (t s e) -> p t e s",
                                         s=2 * K, e=3),
                    axis=mybir.AxisListType.X, op=mybir.AluOpType.add)
                disp = wp.tile([P, nt * 3], F32, tag="disp")
                nc.vector.tensor_tensor(
                    out=disp[:].rearrange("p (t e) -> p t e", e=3),
                    in0=wsum[:].rearrange("p (t e) -> p t e", e=3),
                    in1=ra[:].unsqueeze(2).broadcast_to([P, nt, 3]),
                    op=mybir.AluOpType.mult)
                nc.vector.tensor_tensor(
                    out=out_sb[:, ds(t0 * 3, nt * 3)], in0=disp[:],
                    in1=base_sb[:, ds(t0 * 3, nt * 3)],
                    op=mybir.AluOpType.add)

            with nc.allow_low_precision(
                    reason="fp16 attention intermediates; rel-err budget 2e-2"):
                for ci in range(len(chunks)):
                    emit_compute(ci, emit_gather(ci))

            nc.sync.dma_start(out=out[:, :], in_=out_sb[:])

    nc.finalize()
    return nc


def prep_inputs(sampled_points, sampled_x, edge_index_filtered,
                W_concat, b_concat, W_out, b_out, W_q, b_q, W_k, b_k,
                n_total, shard, n_tiles):
    """Host-side layout prep + weight folding. Returns in_maps for 8 cores."""
    P = 128
    valid = n_total // N_CORES
    scale = np.sqrt(np.float32(H // 2), dtype=np.float32) + 1e-6

    Wc = W_concat.astype(np.float64)
    Wo = W_out.astype(np.float64)
    M1 = Wo @ Wc[:, :H]                                    # [3,128]
    M2 = Wo @ Wc[:, H:]                                    # [3,3]
    c0 = b_concat.astype(np.float64) @ Wo.T + b_out.astype(np.float64)

    w1 = np.zeros((P, W1C), np.float64)
    w1[:, 0:64] = W_k.astype(np.float64).T
    w1[:, 64:67] = M1.T
    w1[:, 67:W1C] = W_q.astype(np.float64).T / scale
    brow = np.zeros((1, W1C), np.float64)
    brow[0, 0:64] = b_k.astype(np.float64)
    brow[0, 67:W1C] = b_q.astype(np.float64) / scale

    w1 = w1.astype(np.float16)
    brow_rep = np.repeat(brow.astype(np.float16), P, 0)
    c0_rep = np.repeat(c0[None].astype(np.float32), P, 0)
    m2_rep = np.repeat(M2.reshape(1, 9).astype(np.float32), P, 0)

    dst = np.asarray(edge_index_filtered[1]).reshape(n_total, K)

    in_maps = []
    for r in range(N_CORES):
        rows = slice(r * valid, (r + 1) * valid)
        x_r = np.zeros((shard, H), np.float16)
        x_r[:valid] = sampled_x[rows].astype(np.float16)
        nb_r = np.zeros((shard, K), np.int64)
        nb_r[:valid] = dst[rows]
        pt_r = np.zeros((shard, 3), np.float32)
        pt_r[:valid] = sampled_points[rows].astype(np.float32)

        def swz(a, width):
            return (a.reshape(n_tiles, P, width).transpose(1, 0, 2)
                    .reshape(P, n_tiles * width).copy())

        # gather indices: position (slot = t*K+k, p) -> idx = nbr//2, stored
        # int16 wrapped-16: [16, pos//16] replicated to all 8 partition groups
        nbs = nb_r.reshape(n_tiles, P, K)
        npos = n_tiles * K * P
        stream = np.empty(npos, np.int64)
        pos = np.arange(npos)
        slot, p = pos // P, pos % P
        t_, k_ = slot // K, slot % K
        stream = nbs[t_, p, k_]
        idxw = (stream // 2).astype(np.int16).reshape(-1, 16).T  # [16, npos/16]
        idx_rep = np.tile(idxw, (8, 1))                          # [128, npos/16]

        par = (stream % 2).astype(np.float32)                    # h=1 half
        nz = (stream != 0).astype(np.float32)
        # masks laid out [p, (t k h)]
        pmask = np.zeros((P, n_tiles * K * 2), np.float32)
        pmask[p, (t_ * K + k_) * 2 + 0] = 1.0 - par
        pmask[p, (t_ * K + k_) * 2 + 1] = par
        cmask = pmask.copy()
        cmask[p, (t_ * K + k_) * 2 + 0] *= nz
        cmask[p, (t_ * K + k_) * 2 + 1] *= nz

        in_maps.append({
            "ax": np.ascontiguousarray(x_r.T),
            "pts": swz(pt_r, 3),
            "idx16": np.ascontiguousarray(idx_rep),
            "cmask": cmask.astype(np.float16),
            "pmask": pmask.astype(np.float16),
            "w1": w1,
            "brow": brow_rep,
            "c0r": c0_rep,
            "m2r": m2_rep,
        })
    return in_maps


def assemble_output(results, n_total, n_tiles):
    P = 128
    valid = n_total // N_CORES
    outs = []
    for r in range(N_CORES):
        o = results[r]["out"]
        o = (o.reshape(P, n_tiles, 3).transpose(1, 0, 2)
             .reshape(n_tiles * P, 3)[:valid])
        outs.append(o)
    return np.concatenate(outs, axis=0).astype(np.float32)


_CACHED = {}


def _get_program(n_total, shard, n_tiles):
    key = (n_total, shard, n_tiles)
    if key not in _CACHED:
        _CACHED[key] = build_program(n_total, shard, n_tiles)
    return _CACHED[key]


def kernel(sampled_points, sampled_x, edge_index_filtered,
           W_concat, b_concat, W_out, b_out, W_q, b_q, W_k, b_k):
    n_total = 60000
    n_tiles = 59
    shard = n_tiles * 128
    nc = _get_program(n_total, shard, n_tiles)
    in_maps = prep_inputs(
        np.asarray(sampled_points), np.asarray(sampled_x),
        np.asarray(edge_index_filtered),
        np.asarray(W_concat), np.asarray(b_concat),
        np.asarray(W_out), np.asarray(b_out),
        np.asarray(W_q), np.asarray(b_q),
        np.asarray(W_k), np.asarray(b_k),
        n_total, shard, n_tiles)
    res = run_bass_kernel_spmd(nc, in_maps, list(range(N_CORES)))
    return assemble_output(res.results, n_total, n_tiles)



# revision 18
# speedup vs baseline: 4.5061x; 1.0126x over previous
"""Trainium2 Bass kernel for MeshGNN message passing (8 NeuronCores, SPMD).

Math reformulation (exact): since softmax weights sum to 1 and the output MLP is
linear, fold W_concat/W_out into per-node quantities:
    M1 = W_out @ W_concat[:, :128]   [3,128]
    M2 = W_out @ W_concat[:, 128:]   [3,3]
    c0 = b_concat @ W_out.T + b_out  [3]
    kx[j] = x[j] @ W_k.T + b_k                  (64,)   -> table
    w[j]  = x[j] @ M1.T + p[j] @ M2.T           (3,)    -> table
    q[n]  = (x[n] @ W_q.T + b_q) / scale        (64,)
    scores[n,k] = q[n] . kx[nbr]
    e = exp(scores * (nbr != 0))                         (scores bounded ~±3)
    out[n] = p[n] + (-v[n] + c0) + sum_k e_k * w[nbr] / sum_k e_k,  v = p @ M2.T

Implementation: per-node table rows of 128 fp16 (=256B): [kx(64)|w(3)|pad61].
Rows are fetched with dma_gather in PAIRS (512B, idx = nbr//2 fits int16),
and the correct half is selected arithmetically via host-prepared parity
masks folded into the score/softmax math (multi-row indirect_dma_start is
broken on HW; dma_gather is the production batched-gather path).
Phase 1 computes the fp16 table + q + base per 128-node tile with one matmul;
an AllGather shares the table; phase 2 gathers pairs chunk-wise and runs the
attention on DVE/ACT.
"""

import sys

import numpy as np

sys.path.insert(0, "/opt/trn_rl_repo")

import concourse.bass as bass
import concourse.mybir as mybir
import concourse.tile as tile
from concourse import bacc
from concourse.bass import ds, ts
from concourse.bass_utils import run_bass_kernel_spmd

N_CORES = 8
H = 128
K = 15
DT = mybir.dt
F16 = DT.float16
F32 = DT.float32
I16 = DT.int16

ROW = 128                 # fp16 elems per table row (256B)
PAIR = 2 * ROW            # gather element: two rows (512B)
QC = 64                   # q/k dim
W1C = 131                 # matmul cols: kx(64)|w(3)|q(64)


def build_program(n_total, shard, n_tiles, chunk_tiles=2):
    valid = n_total // N_CORES
    P = 128
    nc = bacc.Bacc(None, debug=False, num_swdge_queues=4)

    ax = nc.declare_dram_parameter("ax", [P, shard], F16, isOutput=False)    # x.T
    pts = nc.declare_dram_parameter("pts", [P, n_tiles * 3], F32, isOutput=False)
    idx16 = nc.declare_dram_parameter("idx16", [P, n_tiles * K * 8], I16,
                                      isOutput=False)
    cmask = nc.declare_dram_parameter("cmask", [P, n_tiles * K * 2], F16,
                                      isOutput=False)
    pmask = nc.declare_dram_parameter("pmask", [P, n_tiles * K * 2], F16,
                                      isOutput=False)
    w1 = nc.declare_dram_parameter("w1", [P, W1C], F16, isOutput=False)
    brow = nc.declare_dram_parameter("brow", [P, W1C], F16, isOutput=False)
    c0r = nc.declare_dram_parameter("c0r", [P, 3], F32, isOutput=False)
    m2r = nc.declare_dram_parameter("m2r", [P, 9], F32, isOutput=False)
    out = nc.declare_dram_parameter("out", [P, n_tiles * 3], F32, isOutput=True)

    with tile.TileContext(nc) as tc:
        with (
            tc.tile_pool(name="persist", bufs=1) as pp,
            tc.tile_pool(name="dram", bufs=1, space="DRAM") as dp,
            tc.tile_pool(name="psum", bufs=4, space="PSUM") as psp,
            tc.tile_pool(name="kxgp", bufs=8) as kxgp,
            tc.tile_pool(name="work", bufs=2) as wp,
        ):
            # ---- persistent SBUF ----
            xT = pp.tile([P, shard], F16)
            pts_sb = pp.tile([P, n_tiles * 3], F32)
            idx_sb = pp.tile([P, n_tiles * K * 8], I16)
            cm_sb = pp.tile([P, n_tiles * K * 2], F16)
            pm_sb = pp.tile([P, n_tiles * K * 2], F16)
            w1_sb = pp.tile([P, W1C], F16)
            br_sb = pp.tile([P, W1C], F16)
            c0_sb = pp.tile([P, 3], F32)
            m2_sb = pp.tile([P, 9], F32)
            q_sb = pp.tile([P, n_tiles * QC], F16)
            base_sb = pp.tile([P, n_tiles * 3], F32)
            out_sb = pp.tile([P, n_tiles * 3], F32)
            stage_sb = kxgp.tile([P, n_tiles * W1C], F16,
                                 tag="kxg")
            tblall_sb = kxgp.tile([P, n_tiles * ROW], F16,
                                  tag="kxg")

            table_pad = dp.tile([shard, ROW], F16, space="DRAM")
            table_full = dp.tile([n_total, ROW], F16, space="DRAM",
                                 addr_space="Shared")

            nc.sync.dma_start(out=xT[:], in_=ax[:, :])
            nc.sync.dma_start(out=pts_sb[:], in_=pts[:, :])
            nc.sync.dma_start(out=idx_sb[:], in_=idx16[:, :])
            nc.sync.dma_start(out=cm_sb[:], in_=cmask[:, :])
            nc.sync.dma_start(out=pm_sb[:], in_=pmask[:, :])
            nc.sync.dma_start(out=w1_sb[:], in_=w1[:, :])
            nc.sync.dma_start(out=br_sb[:], in_=brow[:, :])
            nc.sync.dma_start(out=c0_sb[:], in_=c0r[:, :])
            nc.sync.dma_start(out=m2_sb[:], in_=m2r[:, :])

            nc.vector.memset(tblall_sb[:], 0)

            # ---- phase 1: matmuls -> ACT-staged copies -> batched DVE ----
            # Matmuls for 3 tiles share one PSUM tile (3*131 fp32 = 1572B
            # fits a 2KB bank) so the ACT copy's ~700ns fixed cost is paid
            # once per 3 tiles instead of per tile.
            NT = n_tiles
            t0 = 0
            while t0 < n_tiles:
                g = min(3, n_tiles - t0)
                ps = psp.tile([P, g * W1C], F32, space="PSUM", tag="ps")
                for j in range(g):
                    nc.tensor.matmul(out=ps[:, ts(j, W1C)],
                                     lhsT=xT[:, ts(t0 + j, P)], rhs=w1_sb[:],
                                     start=True, stop=True)
                nc.scalar.copy(out=stage_sb[:, ds(t0 * W1C, g * W1C)],
                               in_=ps[:])
                t0 += g

            st3 = stage_sb[:, 0:NT * W1C].rearrange("p (t c) -> p t c", c=W1C)
            # v = p @ M2.T for all tiles: [P, NT, 3]
            vp_all = pp.tile([P, NT * 9], F32)
            for j in range(3):
                nc.vector.tensor_tensor(
                    out=vp_all[:].rearrange("p (t j i) -> p t j i", j=3, i=3)
                        [:, :, j, :],
                    in0=pts_sb[:].rearrange("p (t i) -> p t i", i=3),
                    in1=m2_sb[:, ds(3 * j, 3)].unsqueeze(1)
                        .broadcast_to([P, NT, 3]),
                    op=mybir.AluOpType.mult)
            v_all = pp.tile([P, NT * 3], F32)
            nc.vector.tensor_reduce(
                out=v_all[:],
                in_=vp_all[:].rearrange("p (t j i) -> p (t j) i", j=3, i=3),
                axis=mybir.AxisListType.X, op=mybir.AluOpType.add)
            v16 = pp.tile([P, NT * 3], F16)
            nc.vector.tensor_copy(out=v16[:], in_=v_all[:])
            tbl3 = tblall_sb[:, 0:NT * ROW].rearrange("p (t e) -> p t e", e=ROW)
            nc.vector.tensor_tensor(
                out=tbl3[:, :, 0:67], in0=st3[:, :, 0:67],
                in1=br_sb[:, 0:67].unsqueeze(1).broadcast_to([P, NT, 67]),
                op=mybir.AluOpType.add)
            nc.vector.tensor_tensor(
                out=tbl3[:, :, 64:67], in0=tbl3[:, :, 64:67],
                in1=v16[:].rearrange("p (t i) -> p t i", i=3),
                op=mybir.AluOpType.add)
            nc.vector.tensor_tensor(
                out=q_sb[:].rearrange("p (t e) -> p t e", e=QC),
                in0=st3[:, :, 67:W1C],
                in1=br_sb[:, 67:W1C].unsqueeze(1).broadcast_to([P, NT, QC]),
                op=mybir.AluOpType.add)
            b1_all = pp.tile([P, NT * 3], F32)
            nc.vector.tensor_tensor(
                out=b1_all[:].rearrange("p (t i) -> p t i", i=3),
                in0=c0_sb[:].unsqueeze(1).broadcast_to([P, NT, 3]),
                in1=v_all[:].rearrange("p (t i) -> p t i", i=3),
                op=mybir.AluOpType.subtract)
            nc.vector.tensor_tensor(
                out=base_sb[:], in0=b1_all[:], in1=pts_sb[:],
                op=mybir.AluOpType.add)

            table_pairs = table_full[:].rearrange("(a two) e -> a (two e)", two=2)
            chunks = []
            t0 = 0
            while t0 < n_tiles:
                chunks.append((t0, min(chunk_tiles, n_tiles - t0)))
                t0 += chunk_tiles

            # one DMA for the whole slice -> single wait for the collective
            nc.sync.dma_start(
                out=table_pad[:].rearrange("(t p) e -> p t e", p=P),
                in_=tblall_sb[:, 0:NT * ROW].rearrange("p (t e) -> p t e", e=ROW))

            # ---- all-gather the fp16 table ----
            nc.gpsimd.collective_compute(
                "AllGather",
                mybir.AluOpType.bypass,
                replica_groups=[list(range(N_CORES))],
                ins=[table_pad[ds(0, valid), :].opt()],
                outs=[table_full[:].opt()],
            )

            # ---- phase 2: pair-gather + attention ----
            def emit_gather(ci):
                t0, nt = chunks[ci]
                nidx = nt * K * P
                kxg = kxgp.tile([P, nt * K * PAIR], F16, tag="kxg")
                nc.gpsimd.dma_gather(
                    kxg[:].rearrange("p (s e) -> p s e", e=PAIR),
                    table_pairs,
                    idx_sb[:, ds(t0 * K * 8, nt * K * 8)],
                    nidx, nidx, PAIR,
                    single_packet=False,
                    queue_num=ci % 4,
                )
                return kxg


            def emit_compute(ci, kxg):
                t0, nt = chunks[ci]
                kx4 = kxg[:].rearrange("p (t s e) -> p t s e",
                                       s=2 * K, e=ROW)
                qc_ap = (q_sb[:, ds(t0 * QC, nt * QC)]
                         .rearrange("p (t e) -> p t e", e=QC)
                         .unsqueeze(2).broadcast_to([P, nt, 2 * K, QC]))
                prod = wp.tile([P, nt * K * 2 * QC], F16, tag="prod")
                pr5 = prod[:].rearrange("p (t s e) -> p t s e",
                                        s=2 * K, e=QC)
                nc.vector.tensor_tensor(out=pr5, in0=kx4[:, :, :, 0:QC],
                                        in1=qc_ap, op=mybir.AluOpType.mult)
                s2a = wp.tile([P, nt * K * 2], F16, tag="s2a")
                nc.vector.tensor_reduce(out=s2a[:], in_=pr5,
                                        axis=mybir.AxisListType.X,
                                        op=mybir.AluOpType.add)
                sm2a = wp.tile([P, nt * K * 2], F16, tag="sm2a")
                nc.vector.tensor_tensor(out=sm2a[:], in0=s2a[:],
                                        in1=cm_sb[:, ds(t0 * K * 2, nt * K * 2)],
                                        op=mybir.AluOpType.mult)
                sma = wp.tile([P, nt * K], F16, tag="sma")
                nc.vector.tensor_reduce(
                    out=sma[:],
                    in_=sm2a[:].rearrange("p (tk h) -> p tk h", h=2),
                    axis=mybir.AxisListType.X, op=mybir.AluOpType.add)
                ea = wp.tile([P, nt * K], F16, tag="ea")
                nc.scalar.activation(out=ea[:], in_=sma[:],
                                     func=mybir.ActivationFunctionType.Exp)
                sea = wp.tile([P, nt], F16, tag="sea")
                nc.vector.tensor_reduce(
                    out=sea[:], in_=ea[:].rearrange("p (t k) -> p t k", k=K),
                    axis=mybir.AxisListType.X, op=mybir.AluOpType.add)
                ra = wp.tile([P, nt], F16, tag="ra")
                nc.vector.reciprocal(out=ra[:], in_=sea[:])
                esel = wp.tile([P, nt * K * 2], F16, tag="esel")
                nc.vector.tensor_tensor(
                    out=esel[:].rearrange("p (tk h) -> p tk h", h=2),
                    in0=pm_sb[:, ds(t0 * K * 2, nt * K * 2)]
                        .rearrange("p (tk h) -> p tk h", h=2),
                    in1=ea[:].unsqueeze(2).broadcast_to([P, nt * K, 2]),
                    op=mybir.AluOpType.mult)
                wpr = wp.tile([P, nt * K * 2 * 3], F16, tag="wpr")
                nc.vector.tensor_tensor(
                    out=wpr[:].rearrange("p (t s e) -> p t s e", s=2 * K, e=3),
                    in0=kx4[:, :, :, QC:QC + 3],
                    in1=esel[:].rearrange("p (t s) -> p t s", s=2 * K)
                        .unsqueeze(3).broadcast_to([P, nt, 2 * K, 3]),
                    op=mybir.AluOpType.mult)
                wsum = wp.tile([P, nt * 3], F16, tag="wsum")
                nc.vector.tensor_reduce(
                    out=wsum[:],
                    in_=wpr[:].rearrange("p (t s e) -> p t e s",
                                         s=2 * K, e=3),
                    axis=mybir.AxisListType.X, op=mybir.AluOpType.add)
                disp = wp.tile([P, nt * 3], F32, tag="disp")
                nc.vector.tensor_tensor(
                    out=disp[:].rearrange("p (t e) -> p t e", e=3),
                    in0=wsum[:].rearrange("p (t e) -> p t e", e=3),
                    in1=ra[:].unsqueeze(2).broadcast_to([P, nt, 3]),
                    op=mybir.AluOpType.mult)
                nc.vector.tensor_tensor(
                    out=out_sb[:, ds(t0 * 3, nt * 3)], in0=disp[:],
                    in1=base_sb[:, ds(t0 * 3, nt * 3)],
                    op=mybir.AluOpType.add)

            with nc.allow_low_precision(
                    reason="fp16 attention intermediates; rel-err budget 2e-2"):
                for ci in range(len(chunks)):
                    emit_compute(ci, emit_gather(ci))

            nc.sync.dma_start(out=out[:, :], in_=out_sb[:])

    nc.finalize()
    return nc


def prep_inputs(sampled_points, sampled_x, edge_index_filtered,
                W_concat, b_concat, W_out, b_out, W_q, b_q, W_k, b_k,
                n_total, shard, n_tiles):
    """Host-side layout prep + weight folding. Returns in_maps for 8 cores."""
    P = 128
    valid = n_total // N_CORES
    scale = np.sqrt(np.float32(H // 2), dtype=np.float32) + 1e-6

    Wc = W_concat.astype(np.float64)
    Wo = W_out.astype(np.float64)
    M1 = Wo @ Wc[:, :H]                                    # [3,128]
    M2 = Wo @ Wc[:, H:]                                    # [3,3]
    c0 = b_concat.astype(np.float64) @ Wo.T + b_out.astype(np.float64)

    w1 = np.zeros((P, W1C), np.float64)
    w1[:, 0:64] = W_k.astype(np.float64).T
    w1[:, 64:67] = M1.T
    w1[:, 67:W1C] = W_q.astype(np.float64).T / scale
    brow = np.zeros((1, W1C), np.float64)
    brow[0, 0:64] = b_k.astype(np.float64)
    brow[0, 67:W1C] = b_q.astype(np.float64) / scale

    w1 = w1.astype(np.float16)
    brow_rep = np.repeat(brow.astype(np.float16), P, 0)
    c0_rep = np.repeat(c0[None].astype(np.float32), P, 0)
    m2_rep = np.repeat(M2.reshape(1, 9).astype(np.float32), P, 0)

    dst = np.asarray(edge_index_filtered[1]).reshape(n_total, K)

    in_maps = []
    for r in range(N_CORES):
        rows = slice(r * valid, (r + 1) * valid)
        x_r = np.zeros((shard, H), np.float16)
        x_r[:valid] = sampled_x[rows].astype(np.float16)
        nb_r = np.zeros((shard, K), np.int64)
        nb_r[:valid] = dst[rows]
        pt_r = np.zeros((shard, 3), np.float32)
        pt_r[:valid] = sampled_points[rows].astype(np.float32)

        def swz(a, width):
            return (a.reshape(n_tiles, P, width).transpose(1, 0, 2)
                    .reshape(P, n_tiles * width).copy())

        # gather indices: position (slot = t*K+k, p) -> idx = nbr//2, stored
        # int16 wrapped-16: [16, pos//16] replicated to all 8 partition groups
        nbs = nb_r.reshape(n_tiles, P, K)
        npos = n_tiles * K * P
        stream = np.empty(npos, np.int64)
        pos = np.arange(npos)
        slot, p = pos // P, pos % P
        t_, k_ = slot // K, slot % K
        stream = nbs[t_, p, k_]
        idxw = (stream // 2).astype(np.int16).reshape(-1, 16).T  # [16, npos/16]
        idx_rep = np.tile(idxw, (8, 1))                          # [128, npos/16]

        par = (stream % 2).astype(np.float32)                    # h=1 half
        nz = (stream != 0).astype(np.float32)
        # masks laid out [p, (t k h)]
        pmask = np.zeros((P, n_tiles * K * 2), np.float32)
        pmask[p, (t_ * K + k_) * 2 + 0] = 1.0 - par
        pmask[p, (t_ * K + k_) * 2 + 1] = par
        cmask = pmask.copy()
        cmask[p, (t_ * K + k_) * 2 + 0] *= nz
        cmask[p, (t_ * K + k_) * 2 + 1] *= nz

        in_maps.append({
            "ax": np.ascontiguousarray(x_r.T),
            "pts": swz(pt_r, 3),
            "idx16": np.ascontiguousarray(idx_rep),
            "cmask": cmask.astype(np.float16),
            "pmask": pmask.astype(np.float16),
            "w1": w1,
            "brow": brow_rep,
            "c0r": c0_rep,
            "m2r": m2_rep,
        })
    return in_maps


def assemble_output(results, n_total, n_tiles):
    P = 128
    valid = n_total // N_CORES
    outs = []
    for r in range(N_CORES):
        o = results[r]["out"]
        o = (o.reshape(P, n_tiles, 3).transpose(1, 0, 2)
             .reshape(n_tiles * P, 3)[:valid])
        outs.append(o)
    return np.concatenate(outs, axis=0).astype(np.float32)


_CACHED = {}


def _get_program(n_total, shard, n_tiles):
    key = (n_total, shard, n_tiles)
    if key not in _CACHED:
        _CACHED[key] = build_program(n_total, shard, n_tiles)
    return _CACHED[key]


def kernel(sampled_points, sampled_x, edge_index_filtered,
           W_concat, b_concat, W_out, b_out, W_q, b_q, W_k, b_k):
    n_total = 60000
    n_tiles = 59
    shard = n_tiles * 128
    nc = _get_program(n_total, shard, n_tiles)
    in_maps = prep_inputs(
        np.asarray(sampled_points), np.asarray(sampled_x),
        np.asarray(edge_index_filtered),
        np.asarray(W_concat), np.asarray(b_concat),
        np.asarray(W_out), np.asarray(b_out),
        np.asarray(W_q), np.asarray(b_q),
        np.asarray(W_k), np.asarray(b_k),
        n_total, shard, n_tiles)
    res = run_bass_kernel_spmd(nc, in_maps, list(range(N_CORES)))
    return assemble_output(res.results, n_total, n_tiles)

